# revision 60
# baseline (speedup 1.0000x reference)
"""Trainium2 Bass kernel for nn_BallPredictorGNN.

The reference model is a 2-layer GAT over (N=20000, E=640000) followed by an
MLP applied to the LAST node only ("ball") -- the output is a single [2]
vector.  Only the ball's 2-hop dependency cone matters:

  layer 2 aggregates at the ball node only            (~24 in-edges)
  layer 1 aggregates at the ball's in-neighbours S2   (~24 nodes, ~784 edges)
  x @ W1 is needed for the sources of those edges

Host side (pure data routing): extract the cone and lay layer-1 edges out on
a [128 partitions x K chunks] grid; each partition serves one destination
node (high-degree destinations get several partitions).  Source features are
replicated per edge-slot into the xT operand; destination features go into
a per-partition block, so a pair of accumulating TensorE matmuls produces
per-edge rows [as+ad | h] directly -- no on-device index math.

Device side highlights vs the naive version:
  * inputs are packed into 7 priority-ordered DRAM tensors spread over the
    two HWDGE queues (sync + scalar) in strict per-queue FIFO order: the 16
    SDMA engines round-robin across queues at packet granularity, so a
    later transfer on another queue would otherwise delay the straggling
    completion increments of the hot one by 1-2us;
  * the loop is software-pipelined (next chunk's projection is enqueued
    before this chunk's merges) so the in-order PE queue never head-of-line
    blocks on ScalarE/VectorE results;
  * the pe-column aggregate lives in its own PSUM bank (PSUM hazards are
    tracked at bank granularity), letting the softmax reciprocal start
    before the last 256-column h merge lands;
  * the tail runs on 32-row tiles (the layer-2 slot count here is 24), relu
    rides the PSUM->SBUF copies after the transposes, and the deferred
    layer-2 softmax division doubles as the final PSUM->SBUF copy.

The same program is replicated SPMD on all 8 NeuronCores (the cone is tiny,
so replication beats sharding + collectives); core 0's output is returned.
"""

import numpy as np

P = 128
Q2 = 32           # padded layer-2 slot count (must be >= n2)
NEG = np.float32(-1e30)
_CACHE = {}


def _ceil(a, b):
    return -(-a // b)


class _Packer:
    """Pack many small [p, w] operands into one [128, W] array, column-wise."""

    def __init__(self, dtype):
        self.cols = []
        self.pos = 0
        self.slots = {}
        self.dtype = dtype

    def add(self, name, arr):
        arr = np.asarray(arr, dtype=np.float32)
        p, w = arr.shape
        full = np.zeros((P, w), np.float32)
        full[:p] = arr
        self.cols.append(full)
        self.slots[name] = (self.pos, self.pos + w)
        self.pos += w

    def finish(self):
        return np.ascontiguousarray(
            np.concatenate(self.cols, axis=1).astype(self.dtype)
        )


def _host_preprocess(inputs):
    import ml_dtypes

    x = np.asarray(inputs["x"], dtype=np.float32)
    ei = np.asarray(inputs["edge_index"]).astype(np.int64)
    N, F = x.shape
    ball = N - 1
    src, dst = ei[0], ei[1]

    # ---- layer-2 edges into the ball: self loop FIRST (slot q=0) ------------
    e2s = np.concatenate([[ball], src[dst == ball]]).astype(np.int64)
    n2 = len(e2s)
    assert n2 <= Q2, f"ball in-neighbourhood too large: {n2}"
    uniq = np.unique(e2s)
    S2 = np.concatenate([[ball], uniq[uniq != ball]]).astype(np.int64)
    m2 = len(S2)
    loc2 = np.full(N, -1, dtype=np.int64)
    loc2[S2] = np.arange(m2)
    s2_loc = loc2[e2s]  # [n2], s2_loc[0] == 0 (ball)

    # ---- layer-1 edge grid: [partition, chunk] ------------------------------
    in_S2 = np.zeros(N, dtype=bool)
    in_S2[S2] = True
    sel1 = in_S2[dst]
    l1s, l1d = src[sel1], loc2[dst[sel1]]
    by_dst = [[v] for v in S2]  # reference adds a self loop to every node
    for s, d in zip(l1s, l1d):
        by_dst[d].append(s)

    K = 2
    while sum(_ceil(len(g), K) for g in by_dst) > P:
        K += 1
    nparts = [_ceil(len(g), K) for g in by_dst]
    assert sum(nparts) <= P

    grid_src = np.zeros((P, K), dtype=np.int64)
    grid_valid = np.zeros((P, K), dtype=bool)
    slotmap = np.full(P, -1, dtype=np.int64)  # partition -> S2 index
    p = 0
    for sidx in range(m2):
        g = by_dst[sidx]
        for gi in range(nparts[sidx]):
            chunk = g[gi * K : (gi + 1) * K]
            for j, s in enumerate(chunk):
                grid_src[p, j] = s
                grid_valid[p, j] = True
            slotmap[p] = sidx
            p += 1

    # per-partition dst features and per-(partition, chunk) src features
    xd = x[S2[np.maximum(slotmap, 0)]]
    xd[slotmap < 0] = 0
    xg = x[grid_src.T.reshape(-1)]          # [K*P, F]
    xg[~grid_valid.T.reshape(-1)] = 0

    admask = np.where(grid_valid, np.float32(0), NEG).astype(np.float32)
    KP = admask.shape[1]

    # P2: layer-1 aggregation + layer-2 gather one-hot.
    # P2[p, q] = 1 iff partition p serves the dst node of layer-2 edge slot q.
    # Padding slots q>=n2 reuse the ball column (nonzero den => no NaN; they
    # are masked out of layer 2 by e2mask).
    # full 128 columns: merge matmuls with 128 output partitions run at
    # 1 col/cycle; 32-partition outputs measured ~1.5x slower.
    s2_locP = np.zeros(P, dtype=np.int64)
    s2_locP[:n2] = s2_loc
    P2 = (slotmap[:, None] == s2_locP[None, :]).astype(np.float32)

    e2mask = np.full((Q2, 1), NEG, np.float32)
    e2mask[:n2] = 0.0

    # ---- dense weight prep (host) -------------------------------------------
    W1 = np.asarray(inputs["W1"], np.float32)  # [F, 256]
    a_src1 = np.asarray(inputs["a_src1"], np.float32)  # [4, 64]
    a_dst1 = np.asarray(inputs["a_dst1"], np.float32)
    H1, C = a_src1.shape
    D1 = H1 * C
    rhsA = np.zeros((F, 4 + D1), np.float32)  # [as-cols | W1]
    rhsB = np.zeros((F, 4), np.float32)  # [ad-cols]
    for h in range(H1):
        blk = W1[:, h * C : (h + 1) * C]
        rhsA[:, h] = blk @ a_src1[h]
        rhsB[:, h] = blk @ a_dst1[h]
    rhsA[:, 4:] = W1

    W2 = np.asarray(inputs["W2"], np.float32)  # [256, 64]
    a_src2 = np.asarray(inputs["a_src2"], np.float32)[0]
    a_dst2 = np.asarray(inputs["a_dst2"], np.float32)[0]
    # rhs2 layout per 128-row half: [as2(1) | W2(64) | ad2(1)]
    rhs2 = np.zeros((D1, 2 + C), np.float32)
    rhs2[:, 0] = W2 @ a_src2
    rhs2[:, 1 : 1 + C] = W2
    rhs2[:, 1 + C] = W2 @ a_dst2

    b1 = np.asarray(inputs["b1"], np.float32)
    b2 = np.asarray(inputs["b2"], np.float32)
    fc1_b = np.asarray(inputs["fc1_b"], np.float32)
    fc2_b = np.asarray(inputs["fc2_b"], np.float32)
    # zb: all biases the softmax-division deferral must commute past are zero
    zb = not (np.any(b1) or np.any(b2) or np.any(fc1_b))
    zfc2 = not np.any(fc2_b)

    bf16 = ml_dtypes.bfloat16

    # the first projection's operands are split across BOTH HWDGE queues
    # (sync + scalar) so the two ~520B/row transfers fly in parallel and
    # the first matmul starts ~1us earlier
    pkh = _Packer(bf16)                        # sync, first
    pkh.add("rhsA", rhsA)                      # 260

    pkhB = _Packer(bf16)                       # scalar, first
    pkhB.add("xT0", xg[:P].T)                  # 128 (chunk-0 src features)
    pkhB.add("rhsB", rhsB)                     # 4
    pkhB.add("xTd", xd.T)                      # 128 (dst features, [F, P])

    # hot pack 2: merge/softmax operands, needed ~0.5us later (sync, second)
    pkh2 = _Packer(bf16)
    pkh2.add("P2", P2)                         # 128
    pkh2.add("admask", admask)                 # K
    pkh2.add("zero", np.zeros((P, 1), np.float32))
    pkh2.add("id32", np.eye(Q2, dtype=np.float32))

    # warm packs: chunk 1 alone (sync, 2nd) then chunks 2-3 (sync, 3rd);
    # everything rides the two HWDGE queues in strict FIFO -- a third
    # concurrent queue measurably worsens the per-transfer completion
    # straggle on the shared SDMA engines
    c_warm = min(KP, 4)
    pkw = _Packer(bf16)
    if KP > 1:
        pkw.add("xT1", xg[P : 2 * P].T)
    else:
        pkw.add("pad", np.zeros((P, 1), np.float32))
    pkw2 = _Packer(bf16)
    for k in range(2, c_warm):
        pkw2.add(f"xT{k}", xg[k * P : (k + 1) * P].T)
    if not pkw2.cols:
        pkw2.add("pad", np.zeros((P, 1), np.float32))

    # cold packs: chunks 4-5 (scalar, 3rd) then chunk 6.. + layer-2
    # constants (scalar, 4th) -- split so the late-loop chunks keep
    # >=1us of completion-straggle slack
    pkc0 = _Packer(bf16)
    for k in range(c_warm, min(KP, 6)):
        pkc0.add(f"xT{k}", xg[k * P : (k + 1) * P].T)
    if not pkc0.cols:
        pkc0.add("pad", np.zeros((P, 1), np.float32))
    pkc = _Packer(bf16)
    for k in range(max(c_warm, 6), KP):
        pkc.add(f"xT{k}", xg[k * P : (k + 1) * P].T)
    pkc.add("rhs2a", rhs2[:P])                 # 66
    pkc.add("rhs2b", rhs2[P:])                 # 66
    pkc.add("ones", np.ones((P, 1), np.float32))
    pkc.add("e2mask", e2mask)

    # f32 pack: MLP weights (gpsimd queue, needed last)
    pkf = _Packer(np.float32)
    pkf.add("fc1w", np.asarray(inputs["fc1_w"], np.float32))
    pkf.add("fc2w", np.asarray(inputs["fc2_w"], np.float32))
    pkf.add("fc1b", fc1_b[:, None])
    pkf.add("fc2b", fc2_b[None, :])
    if not zb:
        pkf.add("b1bc", np.broadcast_to(b1, (Q2, D1)))
        pkf.add("b2col", b2[:, None])

    feed = {"packh": pkh.finish(), "packhB": pkhB.finish(),
            "packh2": pkh2.finish(), "packw": pkw.finish(),
            "packw2": pkw2.finish(), "packc0": pkc0.finish(),
            "packc": pkc.finish(), "packf": pkf.finish()}
    dims = dict(
        F=F, H1=H1, C=C, K=KP, zb=zb, zfc2=zfc2,
        slots_h=tuple(sorted(pkh.slots.items())),
        slots_hB=tuple(sorted(pkhB.slots.items())),
        slots_h2=tuple(sorted(pkh2.slots.items())),
        slots_w=tuple(sorted(pkw.slots.items())),
        slots_w2=tuple(sorted(pkw2.slots.items())),
        slots_c0=tuple(sorted(pkc0.slots.items())),
        slots_c=tuple(sorted(pkc.slots.items())),
        slots_f=tuple(sorted(pkf.slots.items())),
    )
    return feed, dims


def _build(dims):
    from concourse import bacc, mybir, tile

    F = dims["F"]          # 128 input features
    H1 = dims["H1"]        # 4 heads, layer 1
    C = dims["C"]          # 64 channels per head
    D1 = H1 * C            # 256
    G1 = 4 + D1            # 260 = [e-pre (4) | h (256)]
    K = dims["K"]
    zb = dims["zb"]
    zfc2 = dims["zfc2"]
    slots_h = dict(dims["slots_h"])
    slots_hB = dict(dims["slots_hB"])
    slots_h2 = dict(dims["slots_h2"])
    slots_w = dict(dims["slots_w"])
    slots_w2 = dict(dims["slots_w2"])
    slots_c0 = dict(dims["slots_c0"])
    slots_c = dict(dims["slots_c"])
    slots_f = dict(dims["slots_f"])
    WH = max(b for _, b in slots_h.values())
    WHB = max(b for _, b in slots_hB.values())
    WH2 = max(b for _, b in slots_h2.values())
    WW = max(b for _, b in slots_w.values())
    WW2 = max(b for _, b in slots_w2.values())
    WC0 = max(b for _, b in slots_c0.values())
    WC = max(b for _, b in slots_c.values())
    WF = max(b for _, b in slots_f.values())
    f32 = mybir.dt.float32
    bf16 = mybir.dt.bfloat16

    nc = bacc.Bacc("TRN2", target_bir_lowering=False, debug=False)

    ph_d = nc.declare_dram_parameter("packh", [P, WH], bf16, isOutput=False)
    phB_d = nc.declare_dram_parameter("packhB", [P, WHB], bf16, isOutput=False)
    ph2_d = nc.declare_dram_parameter("packh2", [P, WH2], bf16, isOutput=False)
    pw_d = nc.declare_dram_parameter("packw", [P, WW], bf16, isOutput=False)
    pw2_d = nc.declare_dram_parameter("packw2", [P, WW2], bf16, isOutput=False)
    pc0_d = nc.declare_dram_parameter("packc0", [P, WC0], bf16, isOutput=False)
    pc_d = nc.declare_dram_parameter("packc", [P, WC], bf16, isOutput=False)
    pf_d = nc.declare_dram_parameter("packf", [P, WF], f32, isOutput=False)
    out_d = nc.declare_dram_parameter("out", [1, 2], f32, isOutput=True)

    MUL = mybir.AluOpType.mult
    MAX = mybir.AluOpType.max
    ADD = mybir.AluOpType.add
    Copy = mybir.ActivationFunctionType.Copy
    Exp = mybir.ActivationFunctionType.Exp
    Relu = mybir.ActivationFunctionType.Relu
    Prelu = mybir.ActivationFunctionType.Prelu

    with tile.TileContext(nc) as tc:
        with (
            tc.tile_pool(name="const", bufs=1) as cp,
            tc.tile_pool(name="work", bufs=4) as wp,
            tc.tile_pool(name="msgp", bufs=4) as mp,
            tc.tile_pool(name="psum", bufs=1, space="PSUM") as pp,
        ):
            pgp = ap_ = app = tp0 = tp1 = l2p = pp
            # ---------------- input DMAs on 4 queues --------------------------
            # sync (HWDGE) carries what the first matmuls need; scalar (HWDGE)
            # the next chunks; vector the cold chunks + layer-2 constants;
            # gpsimd (SWDGE, slowest) only the MLP weights needed at the end.
            # Strict per-queue FIFO priority: the 16 SDMA engines round-robin
            # across queues at packet granularity, so a later transfer on
            # ANOTHER queue delays the straggling completion increments of
            # the hot one by 1-2us.  Everything therefore rides the two
            # HWDGE queues, hottest first; gpsimd issues no DMAs at all.
            ph_s = cp.tile([P, WH], bf16)
            nc.sync.dma_start(ph_s[:], ph_d[:])
            phB_s = cp.tile([P, WHB], bf16)
            nc.scalar.dma_start(phB_s[:], phB_d[:])
            pw_s = cp.tile([P, WW], bf16)
            nc.sync.dma_start(pw_s[:], pw_d[:])
            ph2_s = cp.tile([P, WH2], bf16)
            nc.scalar.dma_start(ph2_s[:], ph2_d[:])
            pw2_s = cp.tile([P, WW2], bf16)
            nc.sync.dma_start(pw2_s[:], pw2_d[:])
            pc0_s = cp.tile([P, WC0], bf16)
            nc.scalar.dma_start(pc0_s[:], pc0_d[:])
            pc_s = cp.tile([P, WC], bf16)
            nc.scalar.dma_start(pc_s[:], pc_d[:])
            pf_s = cp.tile([P, WF], f32)
            nc.sync.dma_start(pf_s[:], pf_d[:])

            def sh(name, rows=P):
                if name in slots_h:
                    a, b = slots_h[name]
                    return ph_s[:rows, a:b]
                if name in slots_hB:
                    a, b = slots_hB[name]
                    return phB_s[:rows, a:b]
                a, b = slots_h2[name]
                return ph2_s[:rows, a:b]

            def sc(name, rows=P):
                a, b = slots_c[name]
                return pc_s[:rows, a:b]

            def sf(name, rows=P):
                a, b = slots_f[name]
                return pf_s[:rows, a:b]

            def xts(k):
                if k == 0:
                    return sh("xT0")
                if f"xT{k}" in slots_hB or f"xT{k}" in slots_h2:
                    return sh(f"xT{k}")
                if f"xT{k}" in slots_w:
                    a, b = slots_w[f"xT{k}"]
                    return pw_s[:, a:b]
                if f"xT{k}" in slots_w2:
                    a, b = slots_w2[f"xT{k}"]
                    return pw2_s[:, a:b]
                if f"xT{k}" in slots_c0:
                    a, b = slots_c0[f"xT{k}"]
                    return pc0_s[:, a:b]
                a, b = slots_c[f"xT{k}"]
                return pc_s[:, a:b]

            zcol = sh("zero")[:, 0:1]

            # ---------------- layer-1 edge chunks ----------------------------
            # chunk k: project 128 edge slots -> [as+ad | h], then
            # pe = exp(prelu(e) + mask); msg = [h*pe | pe];
            # agg[q, :] += P2 @ msg  (partition merge + L2 gather fused)
            agg = ap_.tile([P, D1], f32, tag="agg")
            # aggP shares a bank ring with tr0 and den (lifetimes disjoint:
            # aggP's reader finishes right before tr0 is written, tr0's
            # before den) -- freeing a bank for a 4th pg buffer below
            aggP = app.tile([P, 4], f32, tag="tr0")
            pgs, msgs = {}, {}

            def proj(k):
                # the last chunk borrows tr1's bank as a 4th pg buffer:
                # with 3 buffers proj_6 would wait for VectorE's mul_3 to
                # free a slot (~0.9us); tr1's real user runs after mul_6
                if k == K - 1:
                    pg = tp1.tile([P, G1], f32, tag="tr1", name=f"pg{k}")
                else:
                    pg = pgp.tile([P, G1], f32, tag="pg", name=f"pg{k}",
                                  bufs=4)
                pgs[k] = pg
                nc.tensor.matmul(
                    out=pg[:], lhsT=xts(k), rhs=sh("rhsA"),
                    start=True, stop=False,
                )
                nc.tensor.matmul(
                    out=pg[:, 0:4], lhsT=sh("xTd"), rhs=sh("rhsB"),
                    start=False, stop=True, skip_group_check=True,
                )

            def acts(k):
                pg = pgs[k]
                el = wp.tile([P, 4], f32, tag="el", name=f"el{k}")
                nc.scalar.activation(el[:], pg[:, 0:4], Prelu, bias=zcol,
                                     alpha=0.2)
                # pe lives in its own tile (not a column range of msg):
                # ScalarE writing into the pool VectorE is hammering costs
                # it ~110ns per op
                pex = wp.tile([P, 4], bf16, tag="pex", name=f"pex{k}")
                msg = mp.tile([P, D1], bf16, tag="msg", name=f"msg{k}")
                msgs[k] = (el, pex, msg)

            def expo(k):
                el, pex, msg = msgs[k]
                nc.scalar.activation(
                    pex[:], el[:], Exp, bias=sh("admask")[:, k : k + 1]
                )

            def mul(k):
                pg = pgs[k]
                el, pex, msg = msgs[k]
                nc.vector.tensor_tensor(
                    out=msg[:].rearrange("p (h c) -> p h c", c=C),
                    in0=pg[:, 4:].rearrange("p (h c) -> p h c", c=C),
                    in1=pex[:].rearrange("p (h o) -> p h o", o=1)
                    .to_broadcast([P, H1, C]),
                    op=MUL,
                )

            def merge(k):
                el, pex, msg = msgs[k]
                # pe merge first (4 cols, needs only the Exp); on the last
                # chunk this lets the reciprocal start while the h merge
                # still waits for the big multiply
                # pe aggregate in its OWN PSUM bank: the framework tracks
                # PSUM hazards at bank granularity, so this lets the
                # reciprocal start as soon as the last pe merge lands,
                # ~0.6us before the last h merge finishes
                nc.tensor.matmul(
                    out=aggP[:], lhsT=sh("P2"), rhs=pex[:],
                    start=(k == 0), stop=(k == K - 1),
                )
                if k < K - 1:
                    nc.tensor.matmul(
                        out=agg[:], lhsT=sh("P2"), rhs=msg[:],
                        start=(k == 0), stop=False,
                    )
                else:
                    # split the last h merge by head-halves: the first
                    # normalisation (and its transpose) starts one
                    # 128-column stream earlier
                    nc.tensor.matmul(
                        out=agg[:, :P], lhsT=sh("P2"), rhs=msg[:, :P],
                        start=False, stop=True, skip_group_check=True,
                    )
                    nc.tensor.matmul(
                        out=agg[:, P:], lhsT=sh("P2"), rhs=msg[:, P:],
                        start=False, stop=True, skip_group_check=True,
                    )

            # software pipeline: the NEXT chunk's projection is emitted
            # before this chunk's merges, so the in-order PE queue never
            # head-of-line blocks on the Exp/multiply results.  Merges are
            # emitted in pairs (one chunk late) so consecutive pairs share
            # the P2 LDWEIGHTS.
            proj(0)
            for k in range(K):
                acts(k)
                expo(k)
                mul(k)
                if k + 1 < K:
                    proj(k + 1)
                if k % 2 == 1:
                    merge(k - 1)
                    merge(k)
            if (K - 1) % 2 == 0:
                merge(K - 1)

            # -------- layer-1 finalize (VectorE only; no PSUM staging) -------
            # agg rows are layer-2 slots directly (q<=32).  h1 = agg_h * rec
            # per head; relu is applied later, inside the PSUM->SBUF copies
            # after the transposes (relu commutes with transpose).
            rec = cp.tile([Q2, H1], f32)
            nc.vector.reciprocal(rec[:], aggP[:Q2, :])
            # two halves so the first transpose starts before the second
            # half of the normalisation finishes; relu rides the
            # PSUM->SBUF copies after the transposes (relu commutes)
            h1 = cp.tile([Q2, D1], bf16)
            HH = H1 // 2
            for half in range(2):
                hs = slice(half * HH * C, (half + 1) * HH * C)
                nc.vector.tensor_tensor(
                    out=h1[:, hs].rearrange("p (h c) -> p h c", c=C),
                    in0=agg[:Q2, hs].rearrange("p (h c) -> p h c", c=C),
                    in1=rec[:, half * HH : (half + 1) * HH]
                    .rearrange("p (h o) -> p h o", o=1)
                    .to_broadcast([Q2, HH, C]),
                    op=MUL,
                )
            if not zb:
                h1b = cp.tile([Q2, D1], f32)
                nc.vector.tensor_add(h1b[:], h1[:], sf("b1bc", Q2))
                h1 = cp.tile([Q2, D1], bf16)
                nc.vector.tensor_copy(h1[:], h1b[:])

            # -------- transpose to [feature, slot] + fused relu --------------
            id32 = sh("id32", Q2)
            tr0 = tp0.tile([P, Q2], bf16, tag="tr0")
            nc.tensor.transpose(
                out=tr0[:], in_=h1[:, :P], identity=id32
            )
            tr1 = tp1.tile([P, Q2], bf16, tag="tr1")
            nc.tensor.transpose(
                out=tr1[:], in_=h1[:, P:], identity=id32
            )
            # both relu-copies on VectorE: ScalarE ops pay ~110ns extra
            # when VectorE is active, and V's queue is free right here
            T0 = cp.tile([P, Q2], bf16)
            nc.vector.tensor_scalar_max(T0[:], tr0[:], 0.0)
            T1 = cp.tile([P, Q2], bf16)
            nc.vector.tensor_scalar_max(T1[:], tr1[:], 0.0)

            # ---------------- layer-2 projection ------------------------------
            # pg2[q, :] = [e2pre(1) | h2(64)]; e2pre = as2[q] + ad2(ball);
            # ball ad2 via stride-0 lhsT broadcast of T's column 0.
            pg2 = l2p.tile([P, 1 + C], f32, tag="l2")
            # l2-tagged PSUM tiles cycle one bank; their natural serial
            # dependencies make the WAR reuse free.  den gets its own bank
            # so its matmul can overlap the msg2 multiply.
            nc.tensor.matmul(
                out=pg2[:Q2, :], lhsT=T0[:], rhs=sc("rhs2a")[:, : 1 + C],
                start=True, stop=False,
            )
            nc.tensor.matmul(
                out=pg2[:Q2, :], lhsT=T1[:], rhs=sc("rhs2b")[:, : 1 + C],
                start=False, stop=False, skip_group_check=True,
            )
            nc.tensor.matmul(
                out=pg2[:Q2, 0:1], lhsT=T0[:, 0:1].to_broadcast([P, Q2]),
                rhs=sc("rhs2a")[:, 1 + C : 2 + C],
                start=False, stop=False, skip_group_check=True,
            )
            nc.tensor.matmul(
                out=pg2[:Q2, 0:1], lhsT=T1[:, 0:1].to_broadcast([P, Q2]),
                rhs=sc("rhs2b")[:, 1 + C : 2 + C],
                start=False, stop=True, skip_group_check=True,
            )

            # ---------------- layer-2 softmax + aggregate ---------------------
            # ScalarE-written tiles stay in the ScalarE-write pool: writing
            # into a pool VectorE is concurrently writing costs ~110ns/op
            # full-width [128,1] activations: a [32,1] act costs ~250-300ns
            # (few lanes); rows 32-127 compute garbage nothing reads
            el2 = wp.tile([P, 1], f32, bufs=1)
            nc.scalar.activation(el2[:], pg2[:, 0:1], Prelu,
                                 bias=zcol, alpha=0.2)
            pe2 = wp.tile([P, 1], bf16, bufs=1)
            nc.scalar.activation(pe2[:], el2[:], Exp, bias=sc("e2mask"))
            msg2 = cp.tile([Q2, C], bf16)
            nc.vector.tensor_tensor(
                out=msg2[:], in0=pg2[:Q2, 1:],
                in1=pe2[:Q2, 0:1].to_broadcast([Q2, C]), op=MUL,
            )
            # den broadcast to C partitions (stride-0 lhsT): the softmax
            # division then folds into the bcr relu below, removing the
            # final rescale from the critical path
            den = tp0.tile([C, 1], f32, tag="tr0")  # reuses tr0 bank (reader long done)
            nc.tensor.matmul(
                out=den[:], lhsT=pe2[:Q2, 0:1].to_broadcast([Q2, C]),
                rhs=sc("ones", Q2), start=True, stop=True,
            )
            recd = cp.tile([C, 1], f32)
            nc.vector.reciprocal(recd[:], den[:])
            if not zb:
                rec2 = cp.tile([1, 1], f32)
                nc.vector.tensor_copy(rec2[:], recd[0:1, :])
            # ---------------- ball column + MLP (division deferred) ----------
            bc = l2p.tile([C, 1], f32, tag="l2")
            nc.tensor.matmul(
                out=bc[:], lhsT=msg2[:], rhs=sc("ones", Q2),
                start=True, stop=True,
            )
            bcr = cp.tile([C, 1], f32)
            if zb:
                nc.vector.tensor_scalar(bcr[:], bc[:], recd[:], 0.0, MUL, MAX)
            else:
                bb = cp.tile([C, 1], f32)
                nc.vector.tensor_scalar(
                    bb[:], bc[:], rec2[0:1, 0:1], None, MUL
                )
                bb2 = cp.tile([C, 1], f32)
                nc.vector.tensor_add(bb2[:], bb[:], sf("b2col", C))
                nc.vector.tensor_scalar_max(bcr[:], bb2[:], 0.0)
            z = l2p.tile([C // 2, 1], f32, tag="l2")
            nc.tensor.matmul(
                out=z[:], lhsT=sf("fc1w", C), rhs=bcr[:], start=True, stop=True
            )
            zr = cp.tile([C // 2, 1], f32)
            if zb:
                nc.vector.tensor_scalar_max(zr[:], z[:], 0.0)
            else:
                nc.scalar.activation(zr[:], z[:], Relu, bias=sf("fc1b", C // 2))
            fin = l2p.tile([1, 2], f32, tag="l2")
            nc.tensor.matmul(
                out=fin[:], lhsT=zr[:], rhs=sf("fc2w", C // 2),
                start=True, stop=True,
            )
            osb = cp.tile([1, 2], f32)
            if zb:
                # division already folded into bcr; plain PSUM->SBUF copy
                nc.vector.tensor_copy(osb[:], fin[0:1, :])
                if not zfc2:
                    osb2 = cp.tile([1, 2], f32)
                    nc.vector.tensor_add(osb2[:], osb[:], sf("fc2b", 1))
                    osb = osb2
            else:
                # division already applied before the MLP in this path
                osb2 = cp.tile([1, 2], f32)
                nc.vector.tensor_add(osb2[:], fin[0:1, :], sf("fc2b", 1))
                osb = osb2
            nc.sync.dma_start(out_d[:], osb[:], single_packet=True)

    nc.compile()
    return nc


def kernel(**inputs):
    from concourse.bass_utils import run_bass_kernel_spmd

    feed, dims = _host_preprocess(inputs)
    key = (dims["F"], dims["H1"], dims["C"], dims["K"], dims["zb"], dims["zfc2"])
    if key not in _CACHE:
        _CACHE[key] = _build(dims)
    nc = _CACHE[key]

    n_cores = 8
    in_maps = [dict(feed) for _ in range(n_cores)]
    res = run_bass_kernel_spmd(nc, in_maps, core_ids=list(range(n_cores)))
    out = np.asarray(res.results[0]["out"], dtype=np.float32).reshape(2)
    return out


# revision 61
# speedup vs baseline: 1.0119x; 1.0119x over previous
"""Trainium2 Bass kernel for nn_BallPredictorGNN.

The reference model is a 2-layer GAT over (N=20000, E=640000) followed by an
MLP applied to the LAST node only ("ball") -- the output is a single [2]
vector.  Only the ball's 2-hop dependency cone matters:

  layer 2 aggregates at the ball node only            (~24 in-edges)
  layer 1 aggregates at the ball's in-neighbours S2   (~24 nodes, ~784 edges)
  x @ W1 is needed for the sources of those edges

Host side (pure data routing): extract the cone and lay layer-1 edges out on
a [128 partitions x K chunks] grid; each partition serves one destination
node (high-degree destinations get several partitions).  Source features are
replicated per edge-slot into the xT operand; destination features go into
a per-partition block, so a pair of accumulating TensorE matmuls produces
per-edge rows [as+ad | h] directly -- no on-device index math.

Device side highlights vs the naive version:
  * inputs are packed into 7 priority-ordered DRAM tensors spread over the
    two HWDGE queues (sync + scalar) in strict per-queue FIFO order: the 16
    SDMA engines round-robin across queues at packet granularity, so a
    later transfer on another queue would otherwise delay the straggling
    completion increments of the hot one by 1-2us;
  * the loop is software-pipelined (next chunk's projection is enqueued
    before this chunk's merges) so the in-order PE queue never head-of-line
    blocks on ScalarE/VectorE results;
  * the pe-column aggregate lives in its own PSUM bank (PSUM hazards are
    tracked at bank granularity), letting the softmax reciprocal start
    before the last 256-column h merge lands;
  * the tail runs on 32-row tiles (the layer-2 slot count here is 24), relu
    rides the PSUM->SBUF copies after the transposes, and the deferred
    layer-2 softmax division doubles as the final PSUM->SBUF copy.

The same program is replicated SPMD on all 8 NeuronCores (the cone is tiny,
so replication beats sharding + collectives); core 0's output is returned.
"""

import numpy as np

P = 128
Q2 = 32           # padded layer-2 slot count (must be >= n2)
NEG = np.float32(-1e30)
_CACHE = {}


def _ceil(a, b):
    return -(-a // b)


class _Packer:
    """Pack many small [p, w] operands into one [128, W] array, column-wise."""

    def __init__(self, dtype):
        self.cols = []
        self.pos = 0
        self.slots = {}
        self.dtype = dtype

    def add(self, name, arr):
        arr = np.asarray(arr, dtype=np.float32)
        p, w = arr.shape
        full = np.zeros((P, w), np.float32)
        full[:p] = arr
        self.cols.append(full)
        self.slots[name] = (self.pos, self.pos + w)
        self.pos += w

    def finish(self):
        return np.ascontiguousarray(
            np.concatenate(self.cols, axis=1).astype(self.dtype)
        )


def _host_preprocess(inputs):
    import ml_dtypes

    x = np.asarray(inputs["x"], dtype=np.float32)
    ei = np.asarray(inputs["edge_index"]).astype(np.int64)
    N, F = x.shape
    ball = N - 1
    src, dst = ei[0], ei[1]

    # ---- layer-2 edges into the ball: self loop FIRST (slot q=0) ------------
    e2s = np.concatenate([[ball], src[dst == ball]]).astype(np.int64)
    n2 = len(e2s)
    assert n2 <= Q2, f"ball in-neighbourhood too large: {n2}"
    uniq = np.unique(e2s)
    S2 = np.concatenate([[ball], uniq[uniq != ball]]).astype(np.int64)
    m2 = len(S2)
    loc2 = np.full(N, -1, dtype=np.int64)
    loc2[S2] = np.arange(m2)
    s2_loc = loc2[e2s]  # [n2], s2_loc[0] == 0 (ball)

    # ---- layer-1 edge grid: [partition, chunk] ------------------------------
    in_S2 = np.zeros(N, dtype=bool)
    in_S2[S2] = True
    sel1 = in_S2[dst]
    l1s, l1d = src[sel1], loc2[dst[sel1]]
    by_dst = [[v] for v in S2]  # reference adds a self loop to every node
    for s, d in zip(l1s, l1d):
        by_dst[d].append(s)

    K = 2
    while sum(_ceil(len(g), K) for g in by_dst) > P:
        K += 1
    nparts = [_ceil(len(g), K) for g in by_dst]
    assert sum(nparts) <= P

    grid_src = np.zeros((P, K), dtype=np.int64)
    grid_valid = np.zeros((P, K), dtype=bool)
    slotmap = np.full(P, -1, dtype=np.int64)  # partition -> S2 index
    p = 0
    for sidx in range(m2):
        g = by_dst[sidx]
        for gi in range(nparts[sidx]):
            chunk = g[gi * K : (gi + 1) * K]
            for j, s in enumerate(chunk):
                grid_src[p, j] = s
                grid_valid[p, j] = True
            slotmap[p] = sidx
            p += 1

    # per-partition dst features and per-(partition, chunk) src features
    xd = x[S2[np.maximum(slotmap, 0)]]
    xd[slotmap < 0] = 0
    xg = x[grid_src.T.reshape(-1)]          # [K*P, F]
    xg[~grid_valid.T.reshape(-1)] = 0

    admask = np.where(grid_valid, np.float32(0), NEG).astype(np.float32)
    KP = admask.shape[1]

    # P2: layer-1 aggregation + layer-2 gather one-hot.
    # P2[p, q] = 1 iff partition p serves the dst node of layer-2 edge slot q.
    # Padding slots q>=n2 reuse the ball column (nonzero den => no NaN; they
    # are masked out of layer 2 by e2mask).
    # full 128 columns: merge matmuls with 128 output partitions run at
    # 1 col/cycle; 32-partition outputs measured ~1.5x slower.
    s2_locP = np.zeros(P, dtype=np.int64)
    s2_locP[:n2] = s2_loc
    P2 = (slotmap[:, None] == s2_locP[None, :]).astype(np.float32)

    e2mask = np.full((Q2, 1), NEG, np.float32)
    e2mask[:n2] = 0.0

    # ---- dense weight prep (host) -------------------------------------------
    W1 = np.asarray(inputs["W1"], np.float32)  # [F, 256]
    a_src1 = np.asarray(inputs["a_src1"], np.float32)  # [4, 64]
    a_dst1 = np.asarray(inputs["a_dst1"], np.float32)
    H1, C = a_src1.shape
    D1 = H1 * C
    rhsA = np.zeros((F, 4 + D1), np.float32)  # [as-cols | W1]
    rhsB = np.zeros((F, 4), np.float32)  # [ad-cols]
    for h in range(H1):
        blk = W1[:, h * C : (h + 1) * C]
        rhsA[:, h] = blk @ a_src1[h]
        rhsB[:, h] = blk @ a_dst1[h]
    rhsA[:, 4:] = W1

    W2 = np.asarray(inputs["W2"], np.float32)  # [256, 64]
    a_src2 = np.asarray(inputs["a_src2"], np.float32)[0]
    a_dst2 = np.asarray(inputs["a_dst2"], np.float32)[0]
    # rhs2 layout per 128-row half: [as2(1) | W2(64) | ad2(1)]
    rhs2 = np.zeros((D1, 2 + C), np.float32)
    rhs2[:, 0] = W2 @ a_src2
    rhs2[:, 1 : 1 + C] = W2
    rhs2[:, 1 + C] = W2 @ a_dst2

    b1 = np.asarray(inputs["b1"], np.float32)
    b2 = np.asarray(inputs["b2"], np.float32)
    fc1_b = np.asarray(inputs["fc1_b"], np.float32)
    fc2_b = np.asarray(inputs["fc2_b"], np.float32)
    # zb: all biases the softmax-division deferral must commute past are zero
    zb = not (np.any(b1) or np.any(b2) or np.any(fc1_b))
    zfc2 = not np.any(fc2_b)

    bf16 = ml_dtypes.bfloat16

    # the first projection's operands are split across BOTH HWDGE queues
    # (sync + scalar) so the two ~520B/row transfers fly in parallel and
    # the first matmul starts ~1us earlier
    pkh = _Packer(bf16)                        # sync, first
    pkh.add("rhsA", rhsA)                      # 260

    pkhB = _Packer(bf16)                       # scalar, first
    pkhB.add("xT0", xg[:P].T)                  # 128 (chunk-0 src features)
    pkhB.add("rhsB", rhsB)                     # 4
    pkhB.add("xTd", xd.T)                      # 128 (dst features, [F, P])

    # hot pack 2: merge/softmax operands, needed ~0.5us later (sync, second)
    pkh2 = _Packer(bf16)
    pkh2.add("P2", P2)                         # 128
    pkh2.add("admask", admask)                 # K
    pkh2.add("zero", np.zeros((P, 1), np.float32))
    pkh2.add("id32", np.eye(Q2, dtype=np.float32))

    # warm packs: chunk 1 alone (sync, 2nd) then chunks 2-3 (sync, 3rd);
    # everything rides the two HWDGE queues in strict FIFO -- a third
    # concurrent queue measurably worsens the per-transfer completion
    # straggle on the shared SDMA engines
    c_warm = min(KP, 4)
    pkw = _Packer(bf16)
    if KP > 1:
        pkw.add("xT1", xg[P : 2 * P].T)
    else:
        pkw.add("pad", np.zeros((P, 1), np.float32))
    pkw2 = _Packer(bf16)
    for k in range(2, c_warm):
        pkw2.add(f"xT{k}", xg[k * P : (k + 1) * P].T)
    if not pkw2.cols:
        pkw2.add("pad", np.zeros((P, 1), np.float32))

    # cold packs: chunks 4-5 (scalar, 3rd) then chunk 6.. + layer-2
    # constants (scalar, 4th) -- split so the late-loop chunks keep
    # >=1us of completion-straggle slack
    pkc0 = _Packer(bf16)
    for k in range(c_warm, min(KP, 6)):
        pkc0.add(f"xT{k}", xg[k * P : (k + 1) * P].T)
    if not pkc0.cols:
        pkc0.add("pad", np.zeros((P, 1), np.float32))
    pkc = _Packer(bf16)
    for k in range(max(c_warm, 6), KP):
        pkc.add(f"xT{k}", xg[k * P : (k + 1) * P].T)
    pkc.add("rhs2a", rhs2[:P])                 # 66
    pkc.add("rhs2b", rhs2[P:])                 # 66
    pkc.add("ones", np.ones((P, 1), np.float32))
    pkc.add("e2mask", e2mask)

    # f32 pack: MLP weights (gpsimd queue, needed last)
    pkf = _Packer(np.float32)
    pkf.add("fc1w", np.asarray(inputs["fc1_w"], np.float32))
    pkf.add("fc2w", np.asarray(inputs["fc2_w"], np.float32))
    pkf.add("fc1b", fc1_b[:, None])
    pkf.add("fc2b", fc2_b[None, :])
    if not zb:
        pkf.add("b1bc", np.broadcast_to(b1, (Q2, D1)))
        pkf.add("b2col", b2[:, None])

    feed = {"packh": pkh.finish(), "packhB": pkhB.finish(),
            "packh2": pkh2.finish(), "packw": pkw.finish(),
            "packw2": pkw2.finish(), "packc0": pkc0.finish(),
            "packc": pkc.finish(), "packf": pkf.finish()}
    dims = dict(
        F=F, H1=H1, C=C, K=KP, zb=zb, zfc2=zfc2,
        slots_h=tuple(sorted(pkh.slots.items())),
        slots_hB=tuple(sorted(pkhB.slots.items())),
        slots_h2=tuple(sorted(pkh2.slots.items())),
        slots_w=tuple(sorted(pkw.slots.items())),
        slots_w2=tuple(sorted(pkw2.slots.items())),
        slots_c0=tuple(sorted(pkc0.slots.items())),
        slots_c=tuple(sorted(pkc.slots.items())),
        slots_f=tuple(sorted(pkf.slots.items())),
    )
    return feed, dims


def _build(dims):
    from concourse import bacc, mybir, tile

    F = dims["F"]          # 128 input features
    H1 = dims["H1"]        # 4 heads, layer 1
    C = dims["C"]          # 64 channels per head
    D1 = H1 * C            # 256
    G1 = 4 + D1            # 260 = [e-pre (4) | h (256)]
    K = dims["K"]
    zb = dims["zb"]
    zfc2 = dims["zfc2"]
    slots_h = dict(dims["slots_h"])
    slots_hB = dict(dims["slots_hB"])
    slots_h2 = dict(dims["slots_h2"])
    slots_w = dict(dims["slots_w"])
    slots_w2 = dict(dims["slots_w2"])
    slots_c0 = dict(dims["slots_c0"])
    slots_c = dict(dims["slots_c"])
    slots_f = dict(dims["slots_f"])
    WH = max(b for _, b in slots_h.values())
    WHB = max(b for _, b in slots_hB.values())
    WH2 = max(b for _, b in slots_h2.values())
    WW = max(b for _, b in slots_w.values())
    WW2 = max(b for _, b in slots_w2.values())
    WC0 = max(b for _, b in slots_c0.values())
    WC = max(b for _, b in slots_c.values())
    WF = max(b for _, b in slots_f.values())
    f32 = mybir.dt.float32
    bf16 = mybir.dt.bfloat16

    nc = bacc.Bacc("TRN2", target_bir_lowering=False, debug=False)

    ph_d = nc.declare_dram_parameter("packh", [P, WH], bf16, isOutput=False)
    phB_d = nc.declare_dram_parameter("packhB", [P, WHB], bf16, isOutput=False)
    ph2_d = nc.declare_dram_parameter("packh2", [P, WH2], bf16, isOutput=False)
    pw_d = nc.declare_dram_parameter("packw", [P, WW], bf16, isOutput=False)
    pw2_d = nc.declare_dram_parameter("packw2", [P, WW2], bf16, isOutput=False)
    pc0_d = nc.declare_dram_parameter("packc0", [P, WC0], bf16, isOutput=False)
    pc_d = nc.declare_dram_parameter("packc", [P, WC], bf16, isOutput=False)
    pf_d = nc.declare_dram_parameter("packf", [P, WF], f32, isOutput=False)
    out_d = nc.declare_dram_parameter("out", [1, 2], f32, isOutput=True)

    MUL = mybir.AluOpType.mult
    MAX = mybir.AluOpType.max
    ADD = mybir.AluOpType.add
    Copy = mybir.ActivationFunctionType.Copy
    Exp = mybir.ActivationFunctionType.Exp
    Relu = mybir.ActivationFunctionType.Relu
    Prelu = mybir.ActivationFunctionType.Prelu

    with tile.TileContext(nc) as tc:
        with (
            tc.tile_pool(name="const", bufs=1) as cp,
            tc.tile_pool(name="work", bufs=4) as wp,
            tc.tile_pool(name="msgp", bufs=4) as mp,
            tc.tile_pool(name="psum", bufs=1, space="PSUM") as pp,
        ):
            pgp = ap_ = app = tp0 = tp1 = l2p = pp
            # ---------------- input DMAs on 4 queues --------------------------
            # sync (HWDGE) carries what the first matmuls need; scalar (HWDGE)
            # the next chunks; vector the cold chunks + layer-2 constants;
            # gpsimd (SWDGE, slowest) only the MLP weights needed at the end.
            # Strict per-queue FIFO priority: the 16 SDMA engines round-robin
            # across queues at packet granularity, so a later transfer on
            # ANOTHER queue delays the straggling completion increments of
            # the hot one by 1-2us.  Everything therefore rides the two
            # HWDGE queues, hottest first; gpsimd issues no DMAs at all.
            ph_s = cp.tile([P, WH], bf16)
            nc.sync.dma_start(ph_s[:], ph_d[:])
            phB_s = cp.tile([P, WHB], bf16)
            nc.scalar.dma_start(phB_s[:], phB_d[:])
            pw_s = cp.tile([P, WW], bf16)
            nc.sync.dma_start(pw_s[:], pw_d[:])
            ph2_s = cp.tile([P, WH2], bf16)
            nc.scalar.dma_start(ph2_s[:], ph2_d[:])
            pw2_s = cp.tile([P, WW2], bf16)
            nc.sync.dma_start(pw2_s[:], pw2_d[:])
            pc0_s = cp.tile([P, WC0], bf16)
            nc.scalar.dma_start(pc0_s[:], pc0_d[:])
            pc_s = cp.tile([P, WC], bf16)
            nc.scalar.dma_start(pc_s[:], pc_d[:])
            pf_s = cp.tile([P, WF], f32)
            nc.sync.dma_start(pf_s[:], pf_d[:])

            def sh(name, rows=P):
                if name in slots_h:
                    a, b = slots_h[name]
                    return ph_s[:rows, a:b]
                if name in slots_hB:
                    a, b = slots_hB[name]
                    return phB_s[:rows, a:b]
                a, b = slots_h2[name]
                return ph2_s[:rows, a:b]

            def sc(name, rows=P):
                a, b = slots_c[name]
                return pc_s[:rows, a:b]

            def sf(name, rows=P):
                a, b = slots_f[name]
                return pf_s[:rows, a:b]

            def xts(k):
                if k == 0:
                    return sh("xT0")
                if f"xT{k}" in slots_hB or f"xT{k}" in slots_h2:
                    return sh(f"xT{k}")
                if f"xT{k}" in slots_w:
                    a, b = slots_w[f"xT{k}"]
                    return pw_s[:, a:b]
                if f"xT{k}" in slots_w2:
                    a, b = slots_w2[f"xT{k}"]
                    return pw2_s[:, a:b]
                if f"xT{k}" in slots_c0:
                    a, b = slots_c0[f"xT{k}"]
                    return pc0_s[:, a:b]
                a, b = slots_c[f"xT{k}"]
                return pc_s[:, a:b]

            zcol = sh("zero")[:, 0:1]

            # ---------------- layer-1 edge chunks ----------------------------
            # chunk k: project 128 edge slots -> [as+ad | h], then
            # pe = exp(prelu(e) + mask); msg = [h*pe | pe];
            # agg[q, :] += P2 @ msg  (partition merge + L2 gather fused)
            agg = ap_.tile([P, D1], f32, tag="agg")
            # aggP shares a bank ring with tr0 and den (lifetimes disjoint:
            # aggP's reader finishes right before tr0 is written, tr0's
            # before den) -- freeing a bank for a 4th pg buffer below
            aggP = app.tile([P, 4], f32, tag="tr0")
            pgs, msgs = {}, {}

            def proj(k):
                # the last chunk borrows tr1's bank as a 4th pg buffer:
                # with 3 buffers proj_6 would wait for VectorE's mul_3 to
                # free a slot (~0.9us); tr1's real user runs after mul_6
                if k == K - 1:
                    pg = tp1.tile([P, G1], f32, tag="tr1", name=f"pg{k}")
                else:
                    pg = pgp.tile([P, G1], f32, tag="pg", name=f"pg{k}",
                                  bufs=4)
                pgs[k] = pg
                nc.tensor.matmul(
                    out=pg[:], lhsT=xts(k), rhs=sh("rhsA"),
                    start=True, stop=False,
                )
                nc.tensor.matmul(
                    out=pg[:, 0:4], lhsT=sh("xTd"), rhs=sh("rhsB"),
                    start=False, stop=True, skip_group_check=True,
                )

            def acts(k):
                pg = pgs[k]
                el = wp.tile([P, 4], f32, tag="el", name=f"el{k}")
                nc.scalar.activation(el[:], pg[:, 0:4], Prelu, bias=zcol,
                                     alpha=0.2)
                # pe lives in its own tile (not a column range of msg):
                # ScalarE writing into the pool VectorE is hammering costs
                # it ~110ns per op
                pex = wp.tile([P, 4], bf16, tag="pex", name=f"pex{k}")
                msg = mp.tile([P, D1], bf16, tag="msg", name=f"msg{k}")
                msgs[k] = (el, pex, msg)

            def expo(k):
                el, pex, msg = msgs[k]
                nc.scalar.activation(
                    pex[:], el[:], Exp, bias=sh("admask")[:, k : k + 1]
                )

            def mul(k):
                pg = pgs[k]
                el, pex, msg = msgs[k]
                nc.vector.tensor_tensor(
                    out=msg[:].rearrange("p (h c) -> p h c", c=C),
                    in0=pg[:, 4:].rearrange("p (h c) -> p h c", c=C),
                    in1=pex[:].rearrange("p (h o) -> p h o", o=1)
                    .to_broadcast([P, H1, C]),
                    op=MUL,
                )

            def merge(k):
                el, pex, msg = msgs[k]
                # pe merge first (4 cols, needs only the Exp); on the last
                # chunk this lets the reciprocal start while the h merge
                # still waits for the big multiply
                # pe aggregate in its OWN PSUM bank: the framework tracks
                # PSUM hazards at bank granularity, so this lets the
                # reciprocal start as soon as the last pe merge lands,
                # ~0.6us before the last h merge finishes
                nc.tensor.matmul(
                    out=aggP[:], lhsT=sh("P2"), rhs=pex[:],
                    start=(k == 0), stop=(k == K - 1),
                )
                if k < K - 1:
                    nc.tensor.matmul(
                        out=agg[:], lhsT=sh("P2"), rhs=msg[:],
                        start=(k == 0), stop=False,
                    )
                else:
                    # split the last h merge by head-halves: the first
                    # normalisation (and its transpose) starts one
                    # 128-column stream earlier
                    nc.tensor.matmul(
                        out=agg[:, :P], lhsT=sh("P2"), rhs=msg[:, :P],
                        start=False, stop=True, skip_group_check=True,
                    )
                    nc.tensor.matmul(
                        out=agg[:, P:], lhsT=sh("P2"), rhs=msg[:, P:],
                        start=False, stop=True, skip_group_check=True,
                    )

            # software pipeline: the NEXT chunk's projection is emitted
            # before this chunk's merges, so the in-order PE queue never
            # head-of-line blocks on the Exp/multiply results.  Merges are
            # emitted in pairs (one chunk late) so consecutive pairs share
            # the P2 LDWEIGHTS.
            proj(0)
            for k in range(K):
                acts(k)
                expo(k)
                mul(k)
                if k + 1 < K:
                    proj(k + 1)
                if k % 2 == 1:
                    merge(k - 1)
                    merge(k)
            if (K - 1) % 2 == 0:
                merge(K - 1)

            # -------- layer-1 finalize (VectorE only; no PSUM staging) -------
            # agg rows are layer-2 slots directly (q<=32).  h1 = agg_h * rec
            # per head; relu is applied later, inside the PSUM->SBUF copies
            # after the transposes (relu commutes with transpose).
            rec = cp.tile([Q2, H1], f32)
            nc.vector.reciprocal(rec[:], aggP[:Q2, :])
            # two halves so the first transpose starts before the second
            # half of the normalisation finishes; relu rides the
            # PSUM->SBUF copies after the transposes (relu commutes)
            h1 = cp.tile([Q2, D1], bf16)
            HH = H1 // 2
            for half in range(2):
                hs = slice(half * HH * C, (half + 1) * HH * C)
                nc.vector.tensor_tensor(
                    out=h1[:, hs].rearrange("p (h c) -> p h c", c=C),
                    in0=agg[:Q2, hs].rearrange("p (h c) -> p h c", c=C),
                    in1=rec[:, half * HH : (half + 1) * HH]
                    .rearrange("p (h o) -> p h o", o=1)
                    .to_broadcast([Q2, HH, C]),
                    op=MUL,
                )
            if not zb:
                h1b = cp.tile([Q2, D1], f32)
                nc.vector.tensor_add(h1b[:], h1[:], sf("b1bc", Q2))
                h1 = cp.tile([Q2, D1], bf16)
                nc.vector.tensor_copy(h1[:], h1b[:])

            # -------- transpose to [feature, slot] + fused relu --------------
            id32 = sh("id32", Q2)
            tr0 = tp0.tile([P, Q2], bf16, tag="tr0")
            nc.tensor.transpose(
                out=tr0[:], in_=h1[:, :P], identity=id32
            )
            tr1 = tp1.tile([P, Q2], bf16, tag="tr1")
            nc.tensor.transpose(
                out=tr1[:], in_=h1[:, P:], identity=id32
            )
            # both relu-copies on VectorE: ScalarE ops pay ~110ns extra
            # when VectorE is active, and V's queue is free right here
            T0 = cp.tile([P, Q2], bf16)
            nc.vector.tensor_scalar_max(T0[:], tr0[:], 0.0)
            T1 = cp.tile([P, Q2], bf16)
            nc.vector.tensor_scalar_max(T1[:], tr1[:], 0.0)

            # ---------------- layer-2 projection ------------------------------
            # pg2[q, :] = [e2pre(1) | h2(64)]; e2pre = as2[q] + ad2(ball);
            # ball ad2 via stride-0 lhsT broadcast of T's column 0.
            pg2 = l2p.tile([Q2, 1 + C], f32, tag="l2")
            # l2-tagged PSUM tiles cycle one bank; their natural serial
            # dependencies make the WAR reuse free.  den gets its own bank
            # so its matmul can overlap the msg2 multiply.
            nc.tensor.matmul(
                out=pg2[:], lhsT=T0[:], rhs=sc("rhs2a")[:, : 1 + C],
                start=True, stop=False,
            )
            nc.tensor.matmul(
                out=pg2[:], lhsT=T1[:], rhs=sc("rhs2b")[:, : 1 + C],
                start=False, stop=False, skip_group_check=True,
            )
            nc.tensor.matmul(
                out=pg2[:, 0:1], lhsT=T0[:, 0:1].to_broadcast([P, Q2]),
                rhs=sc("rhs2a")[:, 1 + C : 2 + C],
                start=False, stop=False, skip_group_check=True,
            )
            nc.tensor.matmul(
                out=pg2[:, 0:1], lhsT=T1[:, 0:1].to_broadcast([P, Q2]),
                rhs=sc("rhs2b")[:, 1 + C : 2 + C],
                start=False, stop=True, skip_group_check=True,
            )

            # ---------------- layer-2 softmax + aggregate ---------------------
            # ScalarE-written tiles stay in the ScalarE-write pool: writing
            # into a pool VectorE is concurrently writing costs ~110ns/op
            el2 = wp.tile([Q2, 1], f32, bufs=1)
            nc.scalar.activation(el2[:], pg2[:, 0:1], Prelu,
                                 bias=sh("zero", Q2)[:, 0:1], alpha=0.2)
            pe2 = wp.tile([Q2, 1], bf16, bufs=1)
            nc.scalar.activation(pe2[:], el2[:], Exp, bias=sc("e2mask", Q2))
            msg2 = cp.tile([Q2, C], bf16)
            nc.vector.tensor_tensor(
                out=msg2[:], in0=pg2[:, 1:],
                in1=pe2[:, 0:1].to_broadcast([Q2, C]), op=MUL,
            )
            # den broadcast to C partitions (stride-0 lhsT): the softmax
            # division then folds into the bcr relu below, removing the
            # final rescale from the critical path
            den = tp0.tile([C, 1], f32, tag="tr0")  # reuses tr0 bank (reader long done)
            nc.tensor.matmul(
                out=den[:], lhsT=pe2[:, 0:1].to_broadcast([Q2, C]),
                rhs=sc("ones", Q2), start=True, stop=True,
            )
            recd = cp.tile([C, 1], f32)
            nc.vector.reciprocal(recd[:], den[:])
            if not zb:
                rec2 = cp.tile([1, 1], f32)
                nc.vector.tensor_copy(rec2[:], recd[0:1, :])
            # ---------------- ball column + MLP (division deferred) ----------
            bc = l2p.tile([C, 1], f32, tag="l2")
            nc.tensor.matmul(
                out=bc[:], lhsT=msg2[:], rhs=sc("ones", Q2),
                start=True, stop=True,
            )
            bcr = cp.tile([C, 1], f32)
            if zb:
                nc.vector.tensor_scalar(bcr[:], bc[:], recd[:], 0.0, MUL, MAX)
            else:
                bb = cp.tile([C, 1], f32)
                nc.vector.tensor_scalar(
                    bb[:], bc[:], rec2[0:1, 0:1], None, MUL
                )
                bb2 = cp.tile([C, 1], f32)
                nc.vector.tensor_add(bb2[:], bb[:], sf("b2col", C))
                nc.vector.tensor_scalar_max(bcr[:], bb2[:], 0.0)
            z = l2p.tile([C // 2, 1], f32, tag="l2")
            nc.tensor.matmul(
                out=z[:], lhsT=sf("fc1w", C), rhs=bcr[:], start=True, stop=True
            )
            zr = cp.tile([C // 2, 1], f32)
            if zb:
                nc.vector.tensor_scalar_max(zr[:], z[:], 0.0)
            else:
                nc.scalar.activation(zr[:], z[:], Relu, bias=sf("fc1b", C // 2))
            fin = l2p.tile([1, 2], f32, tag="l2")
            nc.tensor.matmul(
                out=fin[:], lhsT=zr[:], rhs=sf("fc2w", C // 2),
                start=True, stop=True,
            )
            osb = cp.tile([1, 2], f32)
            if zb:
                # division already folded into bcr; plain PSUM->SBUF copy
                nc.vector.tensor_copy(osb[:], fin[0:1, :])
                if not zfc2:
                    osb2 = cp.tile([1, 2], f32)
                    nc.vector.tensor_add(osb2[:], osb[:], sf("fc2b", 1))
                    osb = osb2
            else:
                # division already applied before the MLP in this path
                osb2 = cp.tile([1, 2], f32)
                nc.vector.tensor_add(osb2[:], fin[0:1, :], sf("fc2b", 1))
                osb = osb2
            nc.sync.dma_start(out_d[:], osb[:], single_packet=True)

    nc.compile()
    return nc


def kernel(**inputs):
    from concourse.bass_utils import run_bass_kernel_spmd

    feed, dims = _host_preprocess(inputs)
    key = (dims["F"], dims["H1"], dims["C"], dims["K"], dims["zb"], dims["zfc2"])
    if key not in _CACHE:
        _CACHE[key] = _build(dims)
    nc = _CACHE[key]

    n_cores = 8
    in_maps = [dict(feed) for _ in range(n_cores)]
    res = run_bass_kernel_spmd(nc, in_maps, core_ids=list(range(n_cores)))
    out = np.asarray(res.results[0]["out"], dtype=np.float32).reshape(2)
    return out
